# revision 1
# baseline (speedup 1.0000x reference)
"""Bidirectional batch-GRU over ragged graph sequences on 8 Trainium2 cores.

Sharding: core = dir*4 + block. Cores 0-3 run the forward GRU on graph
blocks of 128; cores 4-7 run the backward GRU on the same blocks with
time-reversed inputs (a forward scan over reversed input == the reverse
scan). All raggedness is carried by host-prepared data (padded transposed
inputs, per-step validity masks, segment-max initial state), so one SPMD
program serves all cores and any lengths.

Device program (per step t, batch g=128 graphs, H=512, gates 3H=1536):
  psum[g, 0:1536] = bias_row + x_t @ W_ih^T (+ h @ W_hh^T for r,z cols)
  psum_nh[g, 512] = bias_nh_row + h @ W_hn^T
  r,z = sigmoid(psum[:, :1024]);  n = tanh(psum[:, 1024:] + r * psum_nh)
  h' = n + z*(h - n);  acc += h' * mask[:, t];  hT = transpose(h') for
  the next step's stationary operand.
Matmuls keep the moving operand at N=512 so float32r streams at full rate.
"""

import os
import numpy as np

os.environ.setdefault("NEURON_RT_RESET_CORES", "1")

import concourse.bacc as bacc
import concourse.mybir as mybir
import concourse.tile as tile
from concourse import bass_utils

F32 = mybir.dt.float32
AF = mybir.ActivationFunctionType
ALU = mybir.AluOpType


def _install_ntff_shim():
    """Make trace=True usable: this image's antenv lacks axon_hooks, and
    run_bass_kernel_spmd hard-imports it when tracing is requested."""
    try:
        import antenv.axon_hooks  # noqa: F401
        return
    except ImportError:
        pass
    try:
        import sys
        import types
        import antenv
        mod = types.ModuleType("antenv.axon_hooks")
        mod._hook = None
        mod.set_axon_ntff_profile_hook = lambda h: setattr(mod, "_hook", h)
        mod.get_axon_ntff_profile_hook = lambda: mod._hook
        sys.modules["antenv.axon_hooks"] = mod
        antenv.axon_hooks = mod
        from trn_agent_boot.trn_boot import _ntff_profile_via_ctypes
        hook = _ntff_profile_via_ctypes("/opt/axon/libaxon_pjrt.so")
        if hook is not None:
            mod.set_axon_ntff_profile_hook(hook)
    except Exception:
        pass


_install_ntff_shim()

B, T, H = 512, 128, 512
G3 = 3 * H            # 1536 gate dims
BPC = 128             # graphs per core
NCORES = 8

# Matmul operand tag: "f32" (safe, 4 cyc/row), "f32r" (full-rate at N>=256,
# reduced-precision multiply), "bf16" (full-rate, lowest precision).
MM_MODE = os.environ.get("GRU_MM_MODE", "f32r")

_CACHE = {}
LAST_RESULTS = None


def _mm_dt():
    return {"f32": F32, "f32r": mybir.dt.float32r,
            "bf16": mybir.dt.bfloat16}[MM_MODE]


def _build_program():
    mm = _mm_dt()
    # transpose runs in plain f32 (f32r producers would be required otherwise)
    tr = mybir.dt.bfloat16 if MM_MODE == "bf16" else F32

    nc = bacc.Bacc("TRN2", target_bir_lowering=False, debug=False,
                   num_devices=NCORES)
    xT = nc.dram_tensor("xT", [128, T * 512], mm, kind="ExternalInput").ap()
    wx = nc.dram_tensor("wx", [512, G3], mm, kind="ExternalInput").ap()
    wh = nc.dram_tensor("wh", [512, G3], mm, kind="ExternalInput").ap()
    brow_a = nc.dram_tensor("brow_a", [1, G3], mm, kind="ExternalInput").ap()
    brow_nh = nc.dram_tensor("brow_nh", [1, H], mm, kind="ExternalInput").ap()
    onesr = nc.dram_tensor("onesr", [1, 128], mm, kind="ExternalInput").ap()
    hT0 = nc.dram_tensor("hT0", [128, 512], mm, kind="ExternalInput").ap()
    h0g = nc.dram_tensor("h0g", [128, 512], F32, kind="ExternalInput").ap()
    msk = nc.dram_tensor("msk", [128, T], F32, kind="ExternalInput").ap()
    ident = nc.dram_tensor("ident", [128, 128], tr, kind="ExternalInput").ap()
    out = nc.dram_tensor("out", [128, 512], F32, kind="ExternalOutput").ap()

    with tile.TileContext(nc) as tc:
        with (
            tc.tile_pool(name="const", bufs=1) as cpool,
            tc.tile_pool(name="xin", bufs=4) as xpool,
            tc.tile_pool(name="gates", bufs=2) as gpool,
            tc.tile_pool(name="state", bufs=2) as spool,
            tc.tile_pool(name="accp", bufs=1) as apool,
            tc.tile_pool(name="pa", bufs=2, space="PSUM") as pa_pool,
            tc.tile_pool(name="pb", bufs=1, space="PSUM") as pb_pool,
            tc.tile_pool(name="pt", bufs=1, space="PSUM") as pt_pool,
        ):
            wx_sb, wh_sb = [], []
            for c in range(4):
                t_ = cpool.tile([128, G3], mm, tag=f"wx{c}")
                nc.sync.dma_start(t_[:], wx[c * 128:(c + 1) * 128, :])
                wx_sb.append(t_)
            for c in range(4):
                t_ = cpool.tile([128, G3], mm, tag=f"wh{c}")
                nc.sync.dma_start(t_[:], wh[c * 128:(c + 1) * 128, :])
                wh_sb.append(t_)
            ba_sb = cpool.tile([1, G3], mm, tag="ba")
            nc.sync.dma_start(ba_sb[:], brow_a[:])
            bnh_sb = cpool.tile([1, H], mm, tag="bnh")
            nc.sync.dma_start(bnh_sb[:], brow_nh[:])
            ones_sb = cpool.tile([1, 128], mm, tag="ones")
            nc.sync.dma_start(ones_sb[:], onesr[:])
            id_sb = cpool.tile([128, 128], tr, tag="ident")
            nc.sync.dma_start(id_sb[:], ident[:])
            msk_sb = cpool.tile([128, T], F32, tag="msk")
            nc.sync.dma_start(msk_sb[:], msk[:])

            acc = apool.tile([128, 512], F32, tag="acc")
            nc.vector.memset(acc[:], 0.0)

            hT_prev = spool.tile([128, 512], mm, tag="hT")
            nc.sync.dma_start(hT_prev[:], hT0[:])
            hg_prev = spool.tile([128, 512], F32, tag="hg")
            nc.sync.dma_start(hg_prev[:], h0g[:])

            for t in range(T):
                x_t = xpool.tile([128, 512], mm, tag="x")
                nc.sync.dma_start(x_t[:], xT[:, t * 512:(t + 1) * 512])
                xr = xpool.tile([128, 512], mm, tag="xr")
                nc.scalar.activation(xr[:], x_t[:], AF.Relu)

                p_a = pa_pool.tile([128, G3], F32, tag="pa")
                p_b = pb_pool.tile([128, 512], F32, tag="pb")

                # x-dependent accumulation groups first (no h dependency).
                for nb in range(3):
                    cols = slice(nb * 512, (nb + 1) * 512)
                    nc.tensor.matmul(p_a[:, cols], ones_sb[:],
                                     ba_sb[:, cols], start=True, stop=False)
                    for c in range(4):
                        gcols = slice(c * 128, (c + 1) * 128)
                        nc.tensor.matmul(p_a[:, cols], xr[:, gcols],
                                         wx_sb[c][:, cols],
                                         start=False,
                                         stop=(nb == 2 and c == 3))
                # h-dependent parts: r,z columns of p_a, then p_b (n_hh).
                for nb in range(2):
                    cols = slice(nb * 512, (nb + 1) * 512)
                    for c in range(4):
                        gcols = slice(c * 128, (c + 1) * 128)
                        nc.tensor.matmul(p_a[:, cols],
                                         hT_prev[:, gcols],
                                         wh_sb[c][:, cols],
                                         start=False, stop=(c == 3))
                nc.tensor.matmul(p_b[:], ones_sb[:],
                                 bnh_sb[:], start=True, stop=False)
                for c in range(4):
                    gcols = slice(c * 128, (c + 1) * 128)
                    nc.tensor.matmul(p_b[:], hT_prev[:, gcols],
                                     wh_sb[c][:, 1024:1536],
                                     start=False, stop=(c == 3))

                rz = gpool.tile([128, 1024], F32, tag="rz")
                nc.scalar.activation(rz[:], p_a[:, 0:1024], AF.Sigmoid)
                t2 = gpool.tile([128, 512], F32, tag="t2")
                nc.vector.tensor_mul(t2[:], rz[:, 0:512], p_b[:])
                t3 = gpool.tile([128, 512], F32, tag="t3")
                nc.vector.tensor_add(t3[:], t2[:], p_a[:, 1024:1536])
                n_sb = gpool.tile([128, 512], F32, tag="n")
                nc.scalar.activation(n_sb[:], t3[:], AF.Tanh)

                d_sb = gpool.tile([128, 512], F32, tag="d")
                nc.vector.tensor_sub(d_sb[:], hg_prev[:], n_sb[:])
                e_sb = gpool.tile([128, 512], F32, tag="e")
                nc.vector.tensor_mul(e_sb[:], rz[:, 512:1024], d_sb[:])
                hg = spool.tile([128, 512], F32, tag="hg")
                nc.vector.tensor_add(hg[:], n_sb[:], e_sb[:])

                # acc += h' * mask[:, t]  (per-partition scalar mask)
                nc.vector.scalar_tensor_tensor(
                    acc[:], hg[:], msk_sb[:, t:t + 1], acc[:],
                    op0=ALU.mult, op1=ALU.add)

                if t + 1 < T:
                    p_t = pt_pool.tile([128, 512], F32, tag="pt")
                    hg_mm = hg
                    if tr != F32:
                        hg_mm = gpool.tile([128, 512], tr, tag="hgmm")
                        nc.vector.tensor_copy(hg_mm[:], hg[:])
                    for c in range(4):
                        gcols = slice(c * 128, (c + 1) * 128)
                        nc.tensor.transpose(p_t[:, gcols],
                                            hg_mm[:, gcols], id_sb[:])
                    hT = spool.tile([128, 512], mm, tag="hT")
                    nc.vector.tensor_copy(hT[:], p_t[:])
                    hT_prev = hT
                hg_prev = hg

            nc.sync.dma_start(out[:], acc[:])

    nc.compile()
    return nc


def _host_prep(h, lengths, bias, w_ih, w_hh, b_ih, b_hh, block, direction,
               starts, h0_all, np_mm):
    """Build one core's input map."""
    gs = block * BPC
    lens = lengths[gs:gs + BPC]
    sts = starts[gs:gs + BPC]

    xpad = np.zeros((T, BPC, H), np.float32)
    mask = np.zeros((BPC, T), np.float32)
    node_rows = np.concatenate(
        [np.arange(sts[j], sts[j] + lens[j]) for j in range(BPC)])
    g_idx = np.repeat(np.arange(BPC), lens)
    pos = np.concatenate([np.arange(lens[j]) for j in range(BPC)])
    t_idx = pos if direction == 0 else (T - 1 - pos)
    xpad[t_idx, g_idx] = h[node_rows] + bias
    if direction == 0:
        mask[g_idx, pos] = 1.0
    else:
        mask[g_idx, T - 1 - pos] = 1.0

    # xT [128, T*512]: row p, col t*512 + c*128 + g  = xpad[t, g, 128c+p]
    xT = np.ascontiguousarray(
        xpad.reshape(T, BPC, 4, 128).transpose(3, 0, 2, 1).reshape(128, T * 512)
    ).astype(np_mm)

    h0 = h0_all[gs:gs + BPC]                                   # [g, H]
    hT0 = np.ascontiguousarray(
        h0.reshape(BPC, 4, 128).transpose(2, 1, 0).reshape(128, 512)
    ).astype(np_mm)
    h0g = np.ascontiguousarray(h0).astype(np.float32)

    brow_a = (b_ih + np.concatenate([b_hh[:1024], np.zeros(512, np.float32)])
              ).reshape(1, G3).astype(np_mm)
    brow_nh = b_hh[1024:].reshape(1, H).astype(np_mm)

    return {
        "xT": xT,
        "wx": np.ascontiguousarray(w_ih.T).astype(np_mm),
        "wh": np.ascontiguousarray(w_hh.T).astype(np_mm),
        "brow_a": brow_a,
        "brow_nh": brow_nh,
        "onesr": np.ones((1, 128), np.float32).astype(np_mm),
        "hT0": hT0,
        "h0g": h0g,
        "msk": mask,
        "ident": np.eye(128, dtype=np.float32).astype(
            np_mm if MM_MODE == "bf16" else np.float32),
    }


def kernel(**inputs):
    global LAST_RESULTS
    h = np.asarray(inputs["h"], np.float32)
    lengths = np.asarray(inputs["lengths"]).astype(np.int64)
    bias = np.asarray(inputs["bias"], np.float32)

    starts = np.concatenate([[0], np.cumsum(lengths)[:-1]]).astype(np.int64)
    h0_all = np.maximum.reduceat(h, starts, axis=0)            # segment max

    if MM_MODE == "bf16":
        import ml_dtypes
        np_mm = ml_dtypes.bfloat16
    else:
        np_mm = np.float32

    if "nc" not in _CACHE:
        _CACHE["nc"] = _build_program()
    nc = _CACHE["nc"]

    wkeys = {0: ("w_ih_f", "w_hh_f", "b_ih_f", "b_hh_f"),
             1: ("w_ih_b", "w_hh_b", "b_ih_b", "b_hh_b")}
    in_maps = []
    for core in range(NCORES):
        direction, block = divmod(core, 4)
        kw, kh, kbi, kbh = wkeys[direction]
        in_maps.append(_host_prep(
            h, lengths, bias,
            np.asarray(inputs[kw], np.float32),
            np.asarray(inputs[kh], np.float32),
            np.asarray(inputs[kbi], np.float32),
            np.asarray(inputs[kbh], np.float32),
            block, direction, starts, h0_all, np_mm))

    res = bass_utils.run_bass_kernel_spmd(nc, in_maps,
                                          core_ids=list(range(NCORES)))
    LAST_RESULTS = res

    out = np.zeros((B, 2 * H), np.float32)
    for core in range(NCORES):
        direction, block = divmod(core, 4)
        gs = block * BPC
        acc = np.asarray(res.results[core]["out"], np.float32)  # [g, H]
        cols = slice(0, H) if direction == 0 else slice(H, 2 * H)
        out[gs:gs + BPC, cols] = acc
    out /= lengths[:, None].astype(np.float32)
    return out



# revision 8
# speedup vs baseline: 1.3224x; 1.3224x over previous
"""Bidirectional batch-GRU over ragged graph sequences on 8 Trainium2 cores.

Sharding: core = dir*4 + block. Cores 0-3 run the forward GRU on graph
blocks of 128; cores 4-7 run the backward GRU on the same blocks with
time-reversed inputs. All raggedness is carried by host-prepared data
(padded transposed inputs pre-activated with relu(h+bias), per-step
validity masks, segment-max initial state), so one SPMD program serves
all cores.

All matmul operands are bf16 (1 cyc/row at any moving width; rel-err
budget validated at ~6e-3 vs the 2e-2 gate). Per step, the PE runs
bias+x-projection of step t+1 inside step t's recurrence window so the
tensor engine never idles and holds its full p-state clock:
  p_a[g, 0:1536] = bias_row + x_t @ W_ih^T (+ h @ W_hh^T for r,z cols)
  p_b[g, 512]    = b_hn_row + h @ W_hn^T
  r,z = sigmoid(p_a[:, :1024]); n = tanh(gxn + r * p_b)
  h' = n + z*(h - n);  acc += h' * mask[:, t];  hT = transpose(h')
The n-gate x-projection is copied out of PSUM early (ACT) so the DVE
chain runs in bf16 fast mode.
"""

import os
import numpy as np

os.environ.setdefault("NEURON_RT_RESET_CORES", "1")

import concourse.bacc as bacc
import concourse.mybir as mybir
import concourse.tile as tile
from concourse import bass_utils

F32 = mybir.dt.float32
BF16 = mybir.dt.bfloat16
AF = mybir.ActivationFunctionType
ALU = mybir.AluOpType


def _install_ntff_shim():
    """Make trace=True usable: this image's antenv lacks axon_hooks, and
    run_bass_kernel_spmd hard-imports it when tracing is requested."""
    try:
        import antenv.axon_hooks  # noqa: F401
        return
    except ImportError:
        pass
    try:
        import sys
        import types
        import antenv
        mod = types.ModuleType("antenv.axon_hooks")
        mod._hook = None
        mod.set_axon_ntff_profile_hook = lambda h: setattr(mod, "_hook", h)
        mod.get_axon_ntff_profile_hook = lambda: mod._hook
        sys.modules["antenv.axon_hooks"] = mod
        antenv.axon_hooks = mod
        from trn_agent_boot.trn_boot import _ntff_profile_via_ctypes
        hook = _ntff_profile_via_ctypes("/opt/axon/libaxon_pjrt.so")
        if hook is not None:
            mod.set_axon_ntff_profile_hook(hook)
    except Exception:
        pass


_install_ntff_shim()

B, T, H = 512, 128, 512
G3 = 3 * H
BPC = 128             # graphs per core
NCORES = 8
MM_MODE = "bf16"

_CACHE = {}
LAST_RESULTS = None


def _build_program():
    nc = bacc.Bacc("TRN2", target_bir_lowering=False, debug=False,
                   num_devices=NCORES)
    xT = nc.dram_tensor("xT", [128, T * 512], BF16, kind="ExternalInput").ap()
    wx = nc.dram_tensor("wx", [512, G3], BF16, kind="ExternalInput").ap()
    wh = nc.dram_tensor("wh", [512, G3], BF16, kind="ExternalInput").ap()
    brow_a = nc.dram_tensor("brow_a", [1, G3], BF16, kind="ExternalInput").ap()
    brow_nh = nc.dram_tensor("brow_nh", [1, H], BF16, kind="ExternalInput").ap()
    onesr = nc.dram_tensor("onesr", [1, 128], BF16, kind="ExternalInput").ap()
    hT0 = nc.dram_tensor("hT0", [128, 512], BF16, kind="ExternalInput").ap()
    h0g = nc.dram_tensor("h0g", [128, 512], BF16, kind="ExternalInput").ap()
    msk = nc.dram_tensor("msk", [128, T], F32, kind="ExternalInput").ap()
    ident = nc.dram_tensor("ident", [128, 128], BF16, kind="ExternalInput").ap()
    out = nc.dram_tensor("out", [128, 512], F32, kind="ExternalOutput").ap()

    with tile.TileContext(nc) as tc:
        with (
            tc.tile_pool(name="const", bufs=1) as cpool,
            tc.tile_pool(name="xin", bufs=4) as xpool,
            tc.tile_pool(name="gates", bufs=2) as gpool,
            tc.tile_pool(name="state", bufs=2) as spool,
            tc.tile_pool(name="accp", bufs=1) as apool,
            tc.tile_pool(name="pa", bufs=2, space="PSUM") as pa_pool,
            tc.tile_pool(name="pb", bufs=1, space="PSUM") as pb_pool,
            tc.tile_pool(name="pt", bufs=1, space="PSUM") as pt_pool,
        ):
            wx_sb, wh_sb = [], []
            for c in range(4):
                t_ = cpool.tile([128, G3], BF16, tag=f"wx{c}", name=f"wxs{c}")
                nc.sync.dma_start(t_[:], wx[c * 128:(c + 1) * 128, :])
                wx_sb.append(t_)
            for c in range(4):
                t_ = cpool.tile([128, G3], BF16, tag=f"wh{c}", name=f"whs{c}")
                nc.sync.dma_start(t_[:], wh[c * 128:(c + 1) * 128, :])
                wh_sb.append(t_)
            ba_sb = cpool.tile([1, G3], BF16, tag="ba")
            nc.sync.dma_start(ba_sb[:], brow_a[:])
            bnh_sb = cpool.tile([1, H], BF16, tag="bnh")
            nc.sync.dma_start(bnh_sb[:], brow_nh[:])
            ones_sb = cpool.tile([1, 128], BF16, tag="ones")
            nc.sync.dma_start(ones_sb[:], onesr[:])
            id_sb = cpool.tile([128, 128], BF16, tag="ident")
            nc.sync.dma_start(id_sb[:], ident[:])
            msk_sb = cpool.tile([128, T], F32, tag="msk")
            nc.sync.dma_start(msk_sb[:], msk[:])

            acc = apool.tile([128, 512], F32, tag="acc")
            nc.vector.memset(acc[:], 0.0)

            hT_prev = spool.tile([128, 512], BF16, tag="hT")
            nc.sync.dma_start(hT_prev[:], hT0[:])
            hg_prev = spool.tile([128, 512], BF16, tag="hg")
            nc.sync.dma_start(hg_prev[:], h0g[:])

            x_tiles = {}

            def fetch_x(t):
                x_t = xpool.tile([128, 512], BF16, tag="x")
                nc.sync.dma_start(x_t[:], xT[:, t * 512:(t + 1) * 512])
                x_tiles[t] = x_t

            pa_tiles = {}

            def emit_bias_x(t, part):
                """part 0: biases + first 4 x-mm. part 1: remaining 8 x-mm."""
                p_a = pa_tiles[t]
                x_t = x_tiles[t]
                if part == 0:
                    for nb in range(3):
                        cols = slice(nb * 512, (nb + 1) * 512)
                        nc.tensor.matmul(p_a[:, cols], ones_sb[:],
                                         ba_sb[:, cols], start=True, stop=False)
                    # n-gate x-projection first so gxn copy can run early
                    nb, cols = 2, slice(1024, 1536)
                    for c in range(4):
                        gcols = slice(c * 128, (c + 1) * 128)
                        nc.tensor.matmul(p_a[:, cols], x_t[:, gcols],
                                         wx_sb[c][:, cols],
                                         start=False, stop=(c == 3))
                else:
                    for nb in range(2):
                        cols = slice(nb * 512, (nb + 1) * 512)
                        for c in range(4):
                            gcols = slice(c * 128, (c + 1) * 128)
                            nc.tensor.matmul(p_a[:, cols], x_t[:, gcols],
                                             wx_sb[c][:, cols],
                                             start=False, stop=False)

            # preamble: x(0) fully projected before the scan starts
            fetch_x(0)
            fetch_x(1)
            pa_tiles[0] = pa_pool.tile([128, G3], F32, tag="pa", name="pa0")
            emit_bias_x(0, 0)
            emit_bias_x(0, 1)

            for t in range(T):
                p_a = pa_tiles[t]
                if t + 2 < T:
                    fetch_x(t + 2)

                # --- recurrent matmuls (wait on hT_prev) ---
                for nb in range(2):
                    cols = slice(nb * 512, (nb + 1) * 512)
                    for c in range(4):
                        gcols = slice(c * 128, (c + 1) * 128)
                        nc.tensor.matmul(p_a[:, cols],
                                         hT_prev[:, gcols],
                                         wh_sb[c][:, cols],
                                         start=False, stop=(c == 3))
                p_b = pb_pool.tile([128, 512], F32, tag="pb")
                nc.tensor.matmul(p_b[:], ones_sb[:],
                                 bnh_sb[:], start=True, stop=False)
                for c in range(4):
                    gcols = slice(c * 128, (c + 1) * 128)
                    nc.tensor.matmul(p_b[:], hT_prev[:, gcols],
                                     wh_sb[c][:, 1024:1536],
                                     start=False, stop=(c == 3))

                # --- ACT chain ---
                gxn = gpool.tile([128, 512], BF16, tag="gxn")
                nc.scalar.activation(gxn[:], p_a[:, 1024:1536], AF.Copy)
                r_sb = gpool.tile([128, 512], BF16, tag="r")
                nc.scalar.activation(r_sb[:], p_a[:, 0:512], AF.Sigmoid)
                z_sb = gpool.tile([128, 512], BF16, tag="z")
                nc.scalar.activation(z_sb[:], p_a[:, 512:1024], AF.Sigmoid)

                # --- x-projection of t+1 fills the PE during the chain ---
                if t + 1 < T:
                    pa_tiles[t + 1] = pa_pool.tile([128, G3], F32, tag="pa",
                                                   name=f"pa{t + 1}")
                    emit_bias_x(t + 1, 0)

                t2 = gpool.tile([128, 512], BF16, tag="t2")
                nc.vector.tensor_mul(t2[:], r_sb[:], p_b[:])
                t3 = gpool.tile([128, 512], BF16, tag="t3")
                nc.vector.tensor_add(t3[:], t2[:], gxn[:])
                n_sb = gpool.tile([128, 512], BF16, tag="n")
                nc.scalar.activation(n_sb[:], t3[:], AF.Tanh)
                d_sb = gpool.tile([128, 512], BF16, tag="d")
                nc.vector.tensor_sub(d_sb[:], hg_prev[:], n_sb[:])
                e_sb = gpool.tile([128, 512], BF16, tag="e")
                nc.vector.tensor_mul(e_sb[:], z_sb[:], d_sb[:])
                hg = spool.tile([128, 512], BF16, tag="hg")
                nc.vector.tensor_add(hg[:], n_sb[:], e_sb[:])

                if t + 1 < T:
                    # transpose lands mid x-stream, right when h' is ready
                    p_t = pt_pool.tile([128, 512], BF16, tag="pt")
                    for c in range(4):
                        gcols = slice(c * 128, (c + 1) * 128)
                        nc.tensor.transpose(p_t[:, gcols],
                                            hg[:, gcols], id_sb[:])
                    emit_bias_x(t + 1, 1)
                    hT = spool.tile([128, 512], BF16, tag="hT")
                    nc.vector.tensor_copy(hT[:], p_t[:])
                    hT_prev = hT

                # acc += h' * mask[:, t]
                nc.vector.scalar_tensor_tensor(
                    acc[:], hg[:], msk_sb[:, t:t + 1], acc[:],
                    op0=ALU.mult, op1=ALU.add)
                hg_prev = hg

            nc.sync.dma_start(out[:], acc[:])

    nc.compile()
    return nc


def _host_prep(msg, lengths, block, direction, starts, h0_all, bf):
    """Build one core's input map."""
    gs = block * BPC
    lens = lengths[gs:gs + BPC]
    sts = starts[gs:gs + BPC]

    xpad = np.zeros((T, BPC, H), np.float32)
    mask = np.zeros((BPC, T), np.float32)
    node_rows = np.concatenate(
        [np.arange(sts[j], sts[j] + lens[j]) for j in range(BPC)])
    g_idx = np.repeat(np.arange(BPC), lens)
    pos = np.concatenate([np.arange(lens[j]) for j in range(BPC)])
    t_idx = pos if direction == 0 else (T - 1 - pos)
    xpad[t_idx, g_idx] = msg[node_rows]
    if direction == 0:
        mask[g_idx, pos] = 1.0
    else:
        mask[g_idx, T - 1 - pos] = 1.0

    # xT [128, T*512]: row p, col t*512 + c*128 + g  = xpad[t, g, 128c+p]
    xT = np.ascontiguousarray(
        xpad.reshape(T, BPC, 4, 128).transpose(3, 0, 2, 1).reshape(128, T * 512)
    ).astype(bf)

    h0 = h0_all[gs:gs + BPC]                                   # [g, H]
    hT0 = np.ascontiguousarray(
        h0.reshape(BPC, 4, 128).transpose(2, 1, 0).reshape(128, 512)
    ).astype(bf)
    h0g = np.ascontiguousarray(h0).astype(bf)

    return {
        "xT": xT,
        "hT0": hT0,
        "h0g": h0g,
        "msk": mask,
    }


def kernel(**inputs):
    global LAST_RESULTS
    import ml_dtypes
    bf = ml_dtypes.bfloat16

    h = np.asarray(inputs["h"], np.float32)
    lengths = np.asarray(inputs["lengths"]).astype(np.int64)
    bias = np.asarray(inputs["bias"], np.float32)

    starts = np.concatenate([[0], np.cumsum(lengths)[:-1]]).astype(np.int64)
    h0_all = np.maximum.reduceat(h, starts, axis=0)            # segment max
    msg = np.maximum(h + bias, 0.0)

    if "nc" not in _CACHE:
        _CACHE["nc"] = _build_program()
    nc = _CACHE["nc"]

    wkeys = {0: ("w_ih_f", "w_hh_f", "b_ih_f", "b_hh_f"),
             1: ("w_ih_b", "w_hh_b", "b_ih_b", "b_hh_b")}
    shared = {}
    for direction in (0, 1):
        kw, kh, kbi, kbh = wkeys[direction]
        w_ih = np.asarray(inputs[kw], np.float32)
        w_hh = np.asarray(inputs[kh], np.float32)
        b_ih = np.asarray(inputs[kbi], np.float32)
        b_hh = np.asarray(inputs[kbh], np.float32)
        shared[direction] = {
            "wx": np.ascontiguousarray(w_ih.T).astype(bf),
            "wh": np.ascontiguousarray(w_hh.T).astype(bf),
            "brow_a": (b_ih + np.concatenate(
                [b_hh[:1024], np.zeros(512, np.float32)])
            ).reshape(1, G3).astype(bf),
            "brow_nh": b_hh[1024:].reshape(1, H).astype(bf),
        }
    ones = np.ones((1, 128), np.float32).astype(bf)
    ident = np.eye(128, dtype=np.float32).astype(bf)

    in_maps = []
    for core in range(NCORES):
        direction, block = divmod(core, 4)
        im = _host_prep(msg, lengths, block, direction, starts, h0_all, bf)
        im.update(shared[direction])
        im["onesr"] = ones
        im["ident"] = ident
        in_maps.append(im)

    res = bass_utils.run_bass_kernel_spmd(nc, in_maps,
                                          core_ids=list(range(NCORES)))
    LAST_RESULTS = res

    out = np.zeros((B, 2 * H), np.float32)
    for core in range(NCORES):
        direction, block = divmod(core, 4)
        gs = block * BPC
        acc = np.asarray(res.results[core]["out"], np.float32)  # [g, H]
        cols = slice(0, H) if direction == 0 else slice(H, 2 * H)
        out[gs:gs + BPC, cols] = acc
    out /= lengths[:, None].astype(np.float32)
    return out
